# revision 16
# baseline (speedup 1.0000x reference)
"""Trainium2 Bass kernel for the pre-LN multi-head attention block.

Sharding: 8 cores = 4 batches x 2 query-row halves, collective-free. Each core
computes all 16 heads for its 512 query rows, with full-T k/v for its batch
(k/v compute duplicated across the 2 cores of a batch).

Per-core scheme (C=1024 channels, T=1024 rows, TQ=512 query rows):
  - everything is bf16 into the PE; PSUM accumulates fp32. Host pre-casts x^T
    and all weights to bf16 and lays the weights out slab-contiguous so each
    weight matrix is ONE [128, 8192] DMA (16KB contiguous per partition).
  - LN stats via bf16 ones-matmuls, column-tiled per 512-col half; the
    mean/rstd rowchain runs on 512-wide rows, with ONE batched Ln and ONE Exp
    per LN phase (minimizes ACT table-set switches); rows are broadcast
    across partitions with K=1 ones-matmuls on the PE then one DVE copy to
    bf16 SBUF; normalize = 2 bf16 DVE ops per chunk-half, half 0 first.
  - q matmuls run j-outer in two 4-output-chunk passes (4 PSUM banks each)
    so they overlap the tail of the x normalize.
  - v bias is folded into the proj bias on the host (bp' = bp + bv @ Wp), so
    v PSUM drains are plain copies; v psum is double-buffered. All v matmuls
    run before attention so attention is exp/ACT-bound.
  - scores^T per head pair = 2 matmuls (K=64 halves) which the PE runs
    concurrently via row-group tiling; exp on ACT over 2-chunk [128, 2048]
    groups (scale=0.125 folded in); p stored bf16.
  - attention is software-pipelined: score/exp groups of pair m interleave
    with the attn@v matmuls of pair m-1, so the PE has av work while exps
    pace the pipeline.
  - attn@v: both heads via 65-col augmented v (ones col -> denominator row);
    denominators: psum row 64 -> SBUF, reciprocal_approx_fast, GpSimd
    partition_broadcast, then the av PSUM drain fuses the 1/den scaling.
  - proj: y^T = Wp^T out^T + bias'; double-buffered psum; host transposes.
"""

from contextlib import ExitStack

import ml_dtypes
import numpy as np

import concourse.bacc as bacc
import concourse.mybir as mybir
import concourse.tile as tile
from concourse.bass_utils import run_bass_kernel_spmd

F32 = mybir.dt.float32
BF16 = mybir.dt.bfloat16
AF = mybir.ActivationFunctionType
OP = mybir.AluOpType

B, T, C = 4, 1024, 1024
H, D = 16, 64
TQ = 512           # query rows per core
NCH = 8            # 128-row chunks of C (or T)
EPS = 1e-5

_CACHE = {}


def _build():
    nc = bacc.Bacc(None, target_bir_lowering=False, debug=False)

    xT_d = nc.declare_dram_parameter("xT", [C, T], BF16, isOutput=False)
    wq_d = nc.declare_dram_parameter("wq", [128, NCH * C], BF16, isOutput=False)
    wk_d = nc.declare_dram_parameter("wk", [128, NCH * C], BF16, isOutput=False)
    wv_d = nc.declare_dram_parameter("wv", [128, NCH * C], BF16, isOutput=False)
    wp_d = nc.declare_dram_parameter("wp", [128, NCH * C], BF16, isOutput=False)
    bq_d = nc.declare_dram_parameter("bq", [C], F32, isOutput=False)
    bk_d = nc.declare_dram_parameter("bk", [C], F32, isOutput=False)
    bp_d = nc.declare_dram_parameter("bp", [C], F32, isOutput=False)
    qg_d = nc.declare_dram_parameter("qg", [C], F32, isOutput=False)
    qb_d = nc.declare_dram_parameter("qb", [C], F32, isOutput=False)
    kg_d = nc.declare_dram_parameter("kg", [C], F32, isOutput=False)
    kb_d = nc.declare_dram_parameter("kb", [C], F32, isOutput=False)
    yT_d = nc.declare_dram_parameter("yT", [C, TQ], F32, isOutput=True)

    with tile.TileContext(nc) as tc, ExitStack() as ctx:
        pool = tc.tile_pool

        const = ctx.enter_context(pool(name="const", bufs=1))
        wqp = ctx.enter_context(pool(name="wqp", bufs=1))
        wkp = ctx.enter_context(pool(name="wkp", bufs=1))
        wvp = ctx.enter_context(pool(name="wvp", bufs=1))
        wpp = ctx.enter_context(pool(name="wpp", bufs=1))
        qsbp = ctx.enter_context(pool(name="qsb", bufs=1))
        ksbp = ctx.enter_context(pool(name="ksb", bufs=1))
        vsbp = ctx.enter_context(pool(name="vsb", bufs=1))
        osbp = ctx.enter_context(pool(name="osb", bufs=1))

        # ============ big-load FIFO: x chunks, then all weights ============
        xz_ctx = ExitStack()
        xzp = xz_ctx.enter_context(pool(name="xz", bufs=1))
        xts = []
        for j in range(NCH):
            t = xzp.tile([128, T], BF16, tag=f"x{j}")
            nc.sync.dma_start(out=t[:, 0:512], in_=xT_d[j * 128:(j + 1) * 128, 0:512])
            nc.sync.dma_start(out=t[:, 512:1024], in_=xT_d[j * 128:(j + 1) * 128, 512:1024])
            xts.append(t)

        wq_sb = wqp.tile([128, NCH * C], BF16)
        nc.sync.dma_start(out=wq_sb, in_=wq_d.ap())
        wk_sb = wkp.tile([128, NCH * C], BF16)
        nc.sync.dma_start(out=wk_sb, in_=wk_d.ap())
        wv_sb = wvp.tile([128, NCH * C], BF16)
        nc.sync.dma_start(out=wv_sb, in_=wv_d.ap())
        wp_sb = wpp.tile([128, NCH * C], BF16)
        nc.sync.dma_start(out=wp_sb, in_=wp_d.ap())

        wq_v = wq_sb.rearrange("p (m j c) -> p m j c", m=NCH, j=NCH)
        wk_v = wk_sb.rearrange("p (m j c) -> p m j c", m=NCH, j=NCH)
        wv_v = wv_sb.rearrange("p (g j c) -> p g j c", g=4, j=NCH)
        wp_v = wp_sb.rearrange("p (j c) -> p j c", j=NCH)

        def vec8(name, d):
            t = const.tile([128, 8], F32, tag=name)
            nc.sync.dma_start(out=t, in_=d.ap().rearrange("(j p) -> p j", p=128))
            return t

        bq8 = vec8("bq8", bq_d)
        bk8 = vec8("bk8", bk_d)
        bp8 = vec8("bp8", bp_d)
        qg8 = vec8("qg8", qg_d)
        qb8 = vec8("qb8", qb_d)
        kg8 = vec8("kg8", kg_d)
        kb8 = vec8("kb8", kb_d)

        # ---- constants ----
        ones_blk = const.tile([128, 128], F32, tag="onesblk")
        nc.vector.memset(ones_blk, 1.0)
        ones1b = const.tile([128, 1], BF16, tag="ones1b")   # value 1/C: the
        # stat matmuls then yield mean / E[x^2] rows directly (1/1024 is
        # exact in bf16)
        nc.vector.memset(ones1b, 1.0 / C)
        ones_row = const.tile([1, 128], BF16, tag="onesrow")
        nc.vector.tensor_copy(out=ones_row, in_=ones_blk[0:1, :])
        eps1 = const.tile([1, 1], F32)
        nc.vector.memset(eps1, EPS)
        scr1 = const.tile([1, 1], F32, tag="scr1")
        # dummy Sqrt at t=0 pre-loads the sqrt ACT table set off the critical path
        nc.scalar.activation(out=scr1, in_=eps1, func=AF.Sqrt, bias=eps1, scale=1.0)

        # persistent activations
        q_sb = qsbp.tile([128, NCH, TQ], BF16)      # q^T, later q-hat
        k_sb = ksbp.tile([128, NCH, T], BF16)       # k^T, later k-hat
        v_sb = vsbp.tile([128, NCH, H * 65], BF16)  # v head-interleaved + ones col
        outT_sb = osbp.tile([128, NCH, TQ], BF16)

        tmp_ctx = ExitStack()
        rows = tmp_ctx.enter_context(pool(name="rows", bufs=2))
        packp = tmp_ctx.enter_context(pool(name="pack", bufs=1))
        mrp = tmp_ctx.enter_context(pool(name="mr", bufs=1))
        sqp = tmp_ctx.enter_context(pool(name="sq", bufs=2))
        qsqp = tmp_ctx.enter_context(pool(name="qsq", bufs=1))

        def rowchain(mu_ap, ex2_ap, n, mu_out, rs_out):
            """From packed mean / E[x^2] PSUM rows [1, n]: mu_out (bf16) and
            rs_out = 1/sqrt(var + eps) (bf16)."""
            nc.vector.tensor_copy(out=mu_out, in_=mu_ap)
            t2 = rows.tile([1, 1024], F32, tag="rt2")
            nc.vector.tensor_tensor(out=t2[:, 0:n], in0=mu_ap, in1=mu_out, op=OP.mult)
            d = rows.tile([1, 1024], F32, tag="rd")
            nc.vector.scalar_tensor_tensor(out=d[:, 0:n], in0=t2[:, 0:n], scalar=-1.0,
                                           in1=ex2_ap, op0=OP.mult, op1=OP.add)
            nc.scalar.activation(out=d[:, 0:n], in_=d[:, 0:n], func=AF.Sqrt,
                                 bias=eps1, scale=1.0)
            rcp = rows.tile([1, 1024], F32, tag="rrcp")
            nc.vector.reciprocal_approx_fast(out=rcp[:, 0:n], in_=d[:, 0:n])
            nc.vector.tensor_copy(out=rs_out, in_=rcp[:, 0:n])

        def bc_half(mu_ap, rs_ap, bc_ps):
            """bc_ps[:, 0:512] = mu broadcast, [:, 512:1024] = rs broadcast."""
            nc.tensor.matmul(bc_ps[:, 0:512], ones_row, mu_ap, start=True, stop=True)
            nc.tensor.matmul(bc_ps[:, 512:1024], ones_row, rs_ap, start=True, stop=True)

        # ================= phase A: x stats, normalize =================
        # PSUM discipline: just two pools are live through phases A/B --
        # qmm (banks for q pass tiles) and kmm (k tiles). All stats /
        # broadcast / warm scratch scribbles into dead regions of the q pass
        # tiles (safe: every region's readers complete before the next
        # writer's accumulation group starts; the tile framework tracks the
        # range-level WAR/RAW dependencies).
        qmm_ctx = ExitStack()
        qmmp = qmm_ctx.enter_context(pool(name="qmm", bufs=1, space="PSUM"))
        kmm_ctx = ExitStack()
        kmmp = kmm_ctx.enter_context(pool(name="kmm", bufs=2, space="PSUM"))
        q_ps1 = qmmp.tile([128, 2048], F32, tag="mm")

        # tiny warm-up matmuls at t~0: absorb PE engine-start latency early
        for _ in range(3):
            nc.tensor.matmul(q_ps1[0:1, 1024:1025], ones1b, ones1b,
                             start=True, stop=True)

        # x stats: mean matmuls first (no dependency on the squares), then
        # the square matmuls -- the means start as soon as chunk 0 lands.
        # All land on partition 0 of q_ps1: mu0|mu1|ex0|ex1 packed.
        xstat = q_ps1
        sqts = []
        for j in range(NCH):
            sqt = sqp.tile([128, T], BF16, tag="sqb")
            nc.vector.tensor_tensor(out=sqt, in0=xts[j], in1=xts[j], op=OP.mult)
            sqts.append(sqt)
        for j in range(NCH):
            st, sp = j == 0, j == NCH - 1
            nc.tensor.matmul(xstat[0:1, 0:512], ones1b, xts[j][:, 0:512],
                             start=st, stop=sp)
            nc.tensor.matmul(xstat[0:1, 512:1024], ones1b, xts[j][:, 512:1024],
                             start=st, stop=sp)
        for j in range(NCH):
            st, sp = j == 0, j == NCH - 1
            nc.tensor.matmul(xstat[0:1, 1024:1536], ones1b, sqts[j][:, 0:512],
                             start=st, stop=sp)
            nc.tensor.matmul(xstat[0:1, 1536:2048], ones1b, sqts[j][:, 512:1024],
                             start=st, stop=sp)

        mux = packp.tile([1, 1024], BF16, tag="mux")   # mu0 | mu1
        rsx = packp.tile([1, 1024], BF16, tag="rsx")   # rs0 | rs1
        rowchain(xstat[0:1, 0:1024], xstat[0:1, 1024:2048], 1024, mux, rsx)

        # warm matmul keyed on mux (written ~1us into the chain): keeps the
        # PE HAM activity window hot through the rowchain stall
        nc.tensor.matmul(q_ps1[:, 1024:1536], ones_row, mux[:, 0:512],
                         start=True, stop=True)

        mrx = mrp.tile([128, 2048], BF16)  # mu0|rs0|mu1|rs1 bf16 broadcast
        for h in range(2):
            bch = q_ps1[:, h * 1024:(h + 1) * 1024]
            bc_half(mux[:, h * 512:(h + 1) * 512], rsx[:, h * 512:(h + 1) * 512], bch)
            nc.vector.tensor_copy(out=mrx[:, h * 1024:h * 1024 + 512], in_=bch[:, 0:512])
            nc.vector.tensor_copy(out=mrx[:, h * 1024 + 512:h * 1024 + 1024],
                                  in_=bch[:, 512:1024])

        # normalize in place, half 0 (query cols) first; q pass-1 matmuls are
        # interleaved j-outer so the PE consumes chunks as they land; half 1
        # runs on GpSimd so it doesn't pace the k matmuls behind DVE work
        for j in range(NCH):
            tz = sqp.tile([128, T], BF16, tag="sqb")
            nc.vector.tensor_tensor(out=tz[:, 0:512], in0=xts[j][:, 0:512],
                                    in1=mrx[:, 0:512], op=OP.subtract)
            nc.vector.tensor_tensor(out=xts[j][:, 0:512], in0=tz[:, 0:512],
                                    in1=mrx[:, 512:1024], op=OP.mult)
            for m in range(4):
                nc.tensor.matmul(q_ps1[:, m * 512:(m + 1) * 512], wq_v[:, m, j, :],
                                 xts[j][:, 0:TQ], start=(j == 0), stop=(j == NCH - 1))
        for j in range(NCH):
            tz = sqp.tile([128, T], BF16, tag="sqb")
            nc.vector.tensor_tensor(out=tz[:, 0:512], in0=xts[j][:, 512:1024],
                                    in1=mrx[:, 1024:1536], op=OP.subtract)
            nc.vector.tensor_tensor(out=xts[j][:, 512:1024], in0=tz[:, 0:512],
                                    in1=mrx[:, 1536:2048], op=OP.mult)

        # ones columns of v (DVE, early; cols disjoint from v drains)
        v_ones_view = v_sb.rearrange("p i (h x) -> p i h x", x=65)[:, :, :, 64:65]
        nc.vector.tensor_copy(out=v_ones_view,
                              in_=ones_blk.rearrange("p (i h x) -> p i h x", i=NCH, h=H))

        # ================= phase B: q pass 2, q-LN, k, k-LN =================
        for m in range(4):
            nc.scalar.activation(out=q_sb[:, m, :], in_=q_ps1[:, m * 512:(m + 1) * 512],
                                 func=AF.Identity, bias=bq8[:, m:m + 1], scale=1.0)
        q_ps2 = qmmp.tile([128, 2048], F32, tag="mm")
        for j in range(NCH):
            for m in range(4):
                nc.tensor.matmul(q_ps2[:, m * 512:(m + 1) * 512], wq_v[:, 4 + m, j, :],
                                 xts[j][:, 0:TQ], start=(j == 0), stop=(j == NCH - 1))
        for m in range(4):
            nc.scalar.activation(out=q_sb[:, 4 + m, :], in_=q_ps2[:, m * 512:(m + 1) * 512],
                                 func=AF.Identity, bias=bq8[:, 4 + m:4 + m + 1], scale=1.0)

        # q squares on DVE; the stat ones-matmuls are deferred into the k
        # loop so the PE is not stalled waiting on q drains
        qsq = []
        for m in range(NCH):
            sqt = qsqp.tile([128, TQ], BF16, tag=f"qsq{m}")
            nc.vector.tensor_tensor(out=sqt, in0=q_sb[:, m, :], in1=q_sb[:, m, :],
                                    op=OP.mult)
            qsq.append(sqt)

        muq = packp.tile([1, 512], BF16, tag="muq")
        rsq = packp.tile([1, 512], BF16, tag="rsq")
        mrq = mrp.tile([128, 1024], BF16, tag="mrq")

        # scribble regions inside q_ps2 (its drains complete early in the k
        # loop): q stats on partition 0 cols 0:1024, k stats cols 1024:2048
        # then 0:512 -- wait, k needs 2048; overlap with q regions is fine
        # because their readers complete before the k-stat writers run.
        qstat_ps = q_ps2
        kstat_ps = q_ps2

        def kstat_mm(m):
            sqt = sqp.tile([128, T], BF16, tag="ksq")
            nc.vector.tensor_tensor(out=sqt, in0=k_sb[:, m, :], in1=k_sb[:, m, :], op=OP.mult)
            st, sp = m == 0, m == NCH - 1
            nc.tensor.matmul(kstat_ps[0:1, 0:512], ones1b, k_sb[:, m, 0:512],
                             start=st, stop=sp)
            nc.tensor.matmul(kstat_ps[0:1, 512:1024], ones1b, k_sb[:, m, 512:1024],
                             start=st, stop=sp)
            nc.tensor.matmul(kstat_ps[0:1, 1024:1536], ones1b, sqt[:, 0:512],
                             start=st, stop=sp)
            nc.tensor.matmul(kstat_ps[0:1, 1536:2048], ones1b, sqt[:, 512:1024],
                             start=st, stop=sp)

        for m in range(NCH):
            k_ps = kmmp.tile([128, T], F32, tag="mm")
            for n in range(2):
                for j in range(NCH):
                    nc.tensor.matmul(k_ps[:, n * 512:(n + 1) * 512], wk_v[:, m, j, :],
                                     xts[j][:, n * 512:(n + 1) * 512],
                                     start=(j == 0), stop=(j == NCH - 1))
            nc.scalar.activation(out=k_sb[:, m, :], in_=k_ps, func=AF.Identity,
                                 bias=bk8[:, m:m + 1], scale=1.0)
            if m == 1:
                # q stat ones-matmuls: q drains/squares are done by now
                for mm_ in range(NCH):
                    st, sp = mm_ == 0, mm_ == NCH - 1
                    nc.tensor.matmul(qstat_ps[0:1, 0:512], ones1b, q_sb[:, mm_, :],
                                     start=st, stop=sp)
                    nc.tensor.matmul(qstat_ps[0:1, 512:1024], ones1b, qsq[mm_],
                                     start=st, stop=sp)
                rowchain(qstat_ps[0:1, 0:512], qstat_ps[0:1, 512:1024], 512, muq, rsq)
            if m == 2:
                bcq0 = q_ps2[:, 512:1536]
                bc_half(muq, rsq, bcq0)
                nc.vector.tensor_copy(out=mrq, in_=bcq0)
            if 2 <= m <= 5:
                # q-hat applies, two per k iteration (keeps ACT from backing
                # up the k drains that release the kmm psum banks)
                for mm_ in range(2 * (m - 2), 2 * (m - 2) + 2):
                    t1 = sqp.tile([128, T], BF16, tag="sqb")
                    nc.vector.tensor_tensor(out=t1[:, 0:TQ], in0=q_sb[:, mm_, :],
                                            in1=mrq[:, 0:512], op=OP.subtract)
                    t2 = sqp.tile([128, T], BF16, tag="sqb")
                    nc.vector.tensor_tensor(out=t2[:, 0:TQ], in0=t1[:, 0:TQ],
                                            in1=mrq[:, 512:1024], op=OP.mult)
                    nc.scalar.activation(out=q_sb[:, mm_, :], in_=t2[:, 0:TQ],
                                         func=AF.Identity, bias=qb8[:, mm_:mm_ + 1],
                                         scale=qg8[:, mm_:mm_ + 1])
            if m >= 3:
                # staggered k stats for chunk m-3 (its drain+square are done)
                kstat_mm(m - 3)
        for m in range(NCH - 3, NCH):
            kstat_mm(m)

        kmm_ctx.close()

        # ================= phase C: v matmuls, then attention =================
        vps_ctx = ExitStack()
        vpsp = vps_ctx.enter_context(pool(name="vps", bufs=2, space="PSUM"))

        muk = packp.tile([1, 1024], BF16, tag="muk")
        rsk = packp.tile([1, 1024], BF16, tag="rsk")
        rowchain(kstat_ps[0:1, 0:1024], kstat_ps[0:1, 1024:2048], 1024, muk, rsk)
        # preload the exp ACT table set now -- the ACT queue reaches this
        # during the v matmuls, well before the attention exps need it
        nc.scalar.activation(out=scr1, in_=eps1, func=AF.Exp, scale=1.0)

        mrk = mrp.tile([128, 2048], BF16, tag="mrk")

        def vgroup(g):
            wvsl = wv_v[:, g]
            for i in range(NCH):
                v_ps = vpsp.tile([128, 256], F32, tag="vps")
                for j in range(NCH):
                    nc.tensor.matmul(v_ps, xts[j][:, i * 128:(i + 1) * 128],
                                     wvsl[:, j, :], start=(j == 0), stop=(j == NCH - 1))
                vout = v_sb.rearrange("p i (h x) -> p i h x", x=65)[:, i, g * 4:(g + 1) * 4, 0:64]
                nc.vector.tensor_copy(out=vout, in_=v_ps.rearrange("p (h x) -> p h x", x=64))

        def khat(m):
            t1 = sqp.tile([128, T], BF16, tag="sqb")
            nc.vector.tensor_tensor(out=t1, in0=k_sb[:, m, :],
                                    in1=mrk[:, 0:1024], op=OP.subtract)
            t2 = sqp.tile([128, T], BF16, tag="sqb")
            nc.vector.tensor_tensor(out=t2, in0=t1,
                                    in1=mrk[:, 1024:2048], op=OP.mult)
            nc.scalar.activation(out=k_sb[:, m, :], in_=t2,
                                 func=AF.Identity, bias=kb8[:, m:m + 1],
                                 scale=kg8[:, m:m + 1])

        vgroup(0)
        # k broadcast, hidden under the v matmuls; mrk packs [mu0|mu1|rs0|rs1]
        for h in range(2):
            bckh = q_ps2[:, h * 1024:(h + 1) * 1024]
            bc_half(muk[:, h * 512:(h + 1) * 512], rsk[:, h * 512:(h + 1) * 512], bckh)
            nc.vector.tensor_copy(out=mrk[:, h * 512:h * 512 + 512], in_=bckh[:, 0:512])
            nc.vector.tensor_copy(out=mrk[:, 1024 + h * 512:1024 + h * 512 + 512],
                                  in_=bckh[:, 512:1024])
        vgroup(1)
        khat(0)
        khat(1)
        vgroup(2)
        khat(2)
        khat(3)
        vgroup(3)
        for m in range(4, NCH):
            khat(m)

        vps_ctx.close()
        qmm_ctx.close()
        tmp_ctx.close()
        xz_ctx.close()

        # ---- attention: software-pipelined pairs ----
        att_ctx = ExitStack()
        pexpp = att_ctx.enter_context(pool(name="pexp", bufs=16))
        denp = att_ctx.enter_context(pool(name="den", bufs=1))
        rcbp = att_ctx.enter_context(pool(name="rcb", bufs=2))
        scp = att_ctx.enter_context(pool(name="sc", bufs=2, space="PSUM"))
        avp = att_ctx.enter_context(pool(name="av", bufs=4, space="PSUM"))

        p_tiles = {}
        av_tiles = {}

        def sc_group(m, i, av_mm=None):
            """scores+exp for pair m, chunk i; av_mm emits the interleaved
            av matmuls of the previous pair."""
            sc_ps = scp.tile([128, 1024], F32, tag="sc")
            nc.tensor.matmul(sc_ps[:, 0:512],
                             k_sb[0:64, m, i * 128:(i + 1) * 128],
                             q_sb[0:64, m, :], start=True, stop=True)
            nc.tensor.matmul(sc_ps[:, 512:1024],
                             k_sb[64:128, m, i * 128:(i + 1) * 128],
                             q_sb[64:128, m, :], start=True, stop=True)
            p_sb = pexpp.tile([128, 1024], BF16, tag="p")
            nc.scalar.activation(out=p_sb, in_=sc_ps, func=AF.Exp, scale=0.125)
            p_tiles[m].append(p_sb)
            if av_mm is not None:
                av_mm(i)

        def av_group_fn(mprev):
            p_list = p_tiles[mprev]
            h0, h1 = 2 * mprev, 2 * mprev + 1
            av0 = avp.tile([65, TQ], F32, tag="av")
            av1 = avp.tile([65, TQ], F32, tag="av")
            av_tiles[mprev] = (av0, av1)

            def av_mm(i):
                st, sp = i == 0, i == NCH - 1
                nc.tensor.matmul(av0, v_sb[:, i, h0 * 65:h0 * 65 + 65],
                                 p_list[i][:, 0:512], start=st, stop=sp)
                nc.tensor.matmul(av1, v_sb[:, i, h1 * 65:h1 * 65 + 65],
                                 p_list[i][:, 512:1024], start=st, stop=sp)
            return av_mm

        def av_drain(mprev):
            av0, av1 = av_tiles.pop(mprev)
            p_tiles.pop(mprev)
            dd = denp.tile([1, 2 * TQ], F32, tag="den")
            nc.vector.tensor_copy(out=dd[:, 0:TQ], in_=av0[64:65, :])
            nc.vector.tensor_copy(out=dd[:, TQ:2 * TQ], in_=av1[64:65, :])
            rt = denp.tile([1, 2 * TQ], F32, tag="rect")
            nc.vector.reciprocal_approx_fast(out=rt, in_=dd)
            rbb = rcbp.tile([64, 2 * TQ], F32, tag="rbb")
            nc.gpsimd.partition_broadcast(rbb, rt)
            nc.vector.tensor_tensor(out=outT_sb[0:64, mprev, :],
                                    in0=av0[0:64, :], in1=rbb[:, 0:TQ], op=OP.mult)
            nc.vector.tensor_tensor(out=outT_sb[64:128, mprev, :],
                                    in0=av1[0:64, :], in1=rbb[:, TQ:2 * TQ], op=OP.mult)

        p_tiles[0] = []
        for i in range(NCH):
            sc_group(0, i)
        for m in range(1, NCH):
            p_tiles[m] = []
            av_mm = av_group_fn(m - 1)
            for i in range(NCH):
                sc_group(m, i, av_mm)
            av_drain(m - 1)
        av_mm = av_group_fn(NCH - 1)
        for i in range(NCH):
            av_mm(i)
        av_drain(NCH - 1)

        att_ctx.close()

        # ================= phase D: proj =================
        youtp = ctx.enter_context(pool(name="yout", bufs=2))
        pjp = ctx.enter_context(pool(name="pj", bufs=2, space="PSUM"))
        for m in range(NCH):
            y_ps = pjp.tile([128, TQ], F32, tag="pj")
            for j in range(NCH):
                nc.tensor.matmul(y_ps, wp_v[:, j, m * 128:(m + 1) * 128], outT_sb[:, j, :],
                                 start=(j == 0), stop=(j == NCH - 1))
            y_sb = youtp.tile([128, TQ], F32, tag="y")
            nc.scalar.activation(out=y_sb, in_=y_ps, func=AF.Identity,
                                 bias=bp8[:, m:m + 1], scale=1.0)
            nc.sync.dma_start(out=yT_d[m * 128:(m + 1) * 128, :], in_=y_sb)

    nc.finalize()
    return nc


def _get_nc():
    if "nc" not in _CACHE:
        _CACHE["nc"] = _build()
    return _CACHE["nc"]


def _lay_w(w, gcols):
    """[C, C] -> [128, 8192] slab-contiguous: A[p, m, j, c'] = w[j*128+p, m*gcols+c']."""
    A = w.reshape(NCH, 128, C // gcols, gcols).transpose(1, 2, 0, 3)
    return np.ascontiguousarray(A.reshape(128, NCH * C))


def _prep_inputs(x, norm_g, norm_b, qkv_w, qkv_b, qln_g, qln_b, kln_g, kln_b, proj_w, proj_b):
    x = np.asarray(x, dtype=np.float32)
    norm_g = np.asarray(norm_g, dtype=np.float32)
    norm_b = np.asarray(norm_b, dtype=np.float32)
    qkv_w = np.asarray(qkv_w, dtype=np.float32)
    qkv_b = np.asarray(qkv_b, dtype=np.float32)
    proj_w = np.asarray(proj_w, dtype=np.float32)
    proj_b = np.asarray(proj_b, dtype=np.float32)

    wfold = norm_g[:, None] * qkv_w                    # [C, 3C]
    bfold = qkv_b + norm_b @ qkv_w                     # [3C]
    wq = np.ascontiguousarray(wfold[:, 0:C])
    wk = np.ascontiguousarray(wfold[:, C:2 * C])
    wv = np.ascontiguousarray(wfold[:, 2 * C:3 * C])
    bq, bk, bv = bfold[0:C].copy(), bfold[C:2 * C].copy(), bfold[2 * C:3 * C].copy()
    # v bias folds through attention (softmax rows sum to 1) into proj bias
    bp = proj_b + bv @ proj_w

    bf16 = ml_dtypes.bfloat16
    wp_lay = np.ascontiguousarray(
        proj_w.reshape(NCH, 128, C).transpose(1, 0, 2).reshape(128, NCH * C))
    common = dict(
        wq=_lay_w(wq, 128).astype(bf16), wk=_lay_w(wk, 128).astype(bf16),
        wv=_lay_w(wv, 256).astype(bf16), wp=wp_lay.astype(bf16),
        bq=bq, bk=bk, bp=bp,
        qg=np.asarray(qln_g, dtype=np.float32).copy(),
        qb=np.asarray(qln_b, dtype=np.float32).copy(),
        kg=np.asarray(kln_g, dtype=np.float32).copy(),
        kb=np.asarray(kln_b, dtype=np.float32).copy(),
    )
    in_maps = []
    for core in range(8):
        b, half = core // 2, core % 2
        xp = np.concatenate([x[b, TQ * half:], x[b, :TQ * half]], axis=0) if half else x[b]
        xT = np.ascontiguousarray(xp.T).astype(bf16)
        in_maps.append(dict(common, xT=xT))
    return in_maps


def kernel(**inputs) -> np.ndarray:
    in_maps = _prep_inputs(**inputs)
    nc = _get_nc()
    res = run_bass_kernel_spmd(nc, in_maps, core_ids=list(range(8)))
    out = np.empty((B, T, C), dtype=np.float32)
    for core in range(8):
        b, half = core // 2, core % 2
        out[b, TQ * half:TQ * half + TQ, :] = res.results[core]["yT"].T
    return out


# revision 18
# speedup vs baseline: 1.0223x; 1.0223x over previous
"""Trainium2 Bass kernel for the pre-LN multi-head attention block.

Sharding: 8 cores = 4 batches x 2 query-row halves, collective-free. Each core
computes all 16 heads for its 512 query rows, with full-T k/v for its batch
(k/v compute duplicated across the 2 cores of a batch).

Per-core scheme (C=1024 channels, T=1024 rows, TQ=512 query rows):
  - everything is bf16 into the PE; PSUM accumulates fp32. Host pre-casts x^T
    and all weights to bf16 and lays the weights out slab-contiguous so each
    weight matrix is ONE [128, 8192] DMA (16KB contiguous per partition).
  - LN stats via bf16 ones-matmuls, column-tiled per 512-col half; the
    mean/rstd rowchain runs on 512-wide rows, with ONE batched Ln and ONE Exp
    per LN phase (minimizes ACT table-set switches); rows are broadcast
    across partitions with K=1 ones-matmuls on the PE then one DVE copy to
    bf16 SBUF; normalize = 2 bf16 DVE ops per chunk-half, half 0 first.
  - q matmuls run j-outer in two 4-output-chunk passes (4 PSUM banks each)
    so they overlap the tail of the x normalize.
  - v bias is folded into the proj bias on the host (bp' = bp + bv @ Wp), so
    v PSUM drains are plain copies; v psum is double-buffered. All v matmuls
    run before attention so attention is exp/ACT-bound.
  - scores^T per head pair = 2 matmuls (K=64 halves) which the PE runs
    concurrently via row-group tiling; exp on ACT over 2-chunk [128, 2048]
    groups (scale=0.125 folded in); p stored bf16.
  - attention is software-pipelined: score/exp groups of pair m interleave
    with the attn@v matmuls of pair m-1, so the PE has av work while exps
    pace the pipeline.
  - attn@v: both heads via 65-col augmented v (ones col -> denominator row);
    denominators: psum row 64 -> SBUF, reciprocal_approx_fast, GpSimd
    partition_broadcast, then the av PSUM drain fuses the 1/den scaling.
  - proj: y^T = Wp^T out^T + bias'; double-buffered psum; host transposes.
"""

from contextlib import ExitStack

import ml_dtypes
import numpy as np

import concourse.bacc as bacc
import concourse.mybir as mybir
import concourse.tile as tile
from concourse.bass_utils import run_bass_kernel_spmd

F32 = mybir.dt.float32
BF16 = mybir.dt.bfloat16
AF = mybir.ActivationFunctionType
OP = mybir.AluOpType

B, T, C = 4, 1024, 1024
H, D = 16, 64
TQ = 512           # query rows per core
NCH = 8            # 128-row chunks of C (or T)
EPS = 1e-5

_CACHE = {}


def _build():
    nc = bacc.Bacc(None, target_bir_lowering=False, debug=False)

    xT_d = nc.declare_dram_parameter("xT", [C, T], BF16, isOutput=False)
    wq_d = nc.declare_dram_parameter("wq", [128, NCH * C], BF16, isOutput=False)
    wk_d = nc.declare_dram_parameter("wk", [128, NCH * C], BF16, isOutput=False)
    wv_d = nc.declare_dram_parameter("wv", [128, NCH * C], BF16, isOutput=False)
    wp_d = nc.declare_dram_parameter("wp", [128, NCH * C], BF16, isOutput=False)
    bq_d = nc.declare_dram_parameter("bq", [C], F32, isOutput=False)
    bk_d = nc.declare_dram_parameter("bk", [C], F32, isOutput=False)
    bp_d = nc.declare_dram_parameter("bp", [C], F32, isOutput=False)
    qg_d = nc.declare_dram_parameter("qg", [C], F32, isOutput=False)
    qb_d = nc.declare_dram_parameter("qb", [C], F32, isOutput=False)
    kg_d = nc.declare_dram_parameter("kg", [C], F32, isOutput=False)
    kb_d = nc.declare_dram_parameter("kb", [C], F32, isOutput=False)
    yT_d = nc.declare_dram_parameter("yT", [C, TQ], F32, isOutput=True)

    with tile.TileContext(nc) as tc, ExitStack() as ctx:
        pool = tc.tile_pool

        const = ctx.enter_context(pool(name="const", bufs=1))
        wqp = ctx.enter_context(pool(name="wqp", bufs=1))
        wkp = ctx.enter_context(pool(name="wkp", bufs=1))
        wvp = ctx.enter_context(pool(name="wvp", bufs=1))
        wpp = ctx.enter_context(pool(name="wpp", bufs=1))
        qsbp = ctx.enter_context(pool(name="qsb", bufs=1))
        ksbp = ctx.enter_context(pool(name="ksb", bufs=1))
        vsbp = ctx.enter_context(pool(name="vsb", bufs=1))
        osbp = ctx.enter_context(pool(name="osb", bufs=1))

        # ============ big-load FIFO: x chunks, then all weights ============
        xz_ctx = ExitStack()
        xzp = xz_ctx.enter_context(pool(name="xz", bufs=1))
        xts = []
        for j in range(NCH):
            t = xzp.tile([128, T], BF16, tag=f"x{j}")
            nc.sync.dma_start(out=t, in_=xT_d[j * 128:(j + 1) * 128, :])
            xts.append(t)

        wq_sb = wqp.tile([128, NCH * C], BF16)
        nc.sync.dma_start(out=wq_sb, in_=wq_d.ap())
        wk_sb = wkp.tile([128, NCH * C], BF16)
        nc.sync.dma_start(out=wk_sb, in_=wk_d.ap())
        wv_sb = wvp.tile([128, NCH * C], BF16)
        nc.sync.dma_start(out=wv_sb, in_=wv_d.ap())
        wp_sb = wpp.tile([128, NCH * C], BF16)
        nc.sync.dma_start(out=wp_sb, in_=wp_d.ap())

        wq_v = wq_sb.rearrange("p (m j c) -> p m j c", m=NCH, j=NCH)
        wk_v = wk_sb.rearrange("p (m j c) -> p m j c", m=NCH, j=NCH)
        wv_v = wv_sb.rearrange("p (g j c) -> p g j c", g=4, j=NCH)
        wp_v = wp_sb.rearrange("p (j c) -> p j c", j=NCH)

        def vec8(name, d):
            t = const.tile([128, 8], F32, tag=name)
            nc.sync.dma_start(out=t, in_=d.ap().rearrange("(j p) -> p j", p=128))
            return t

        bq8 = vec8("bq8", bq_d)
        bk8 = vec8("bk8", bk_d)
        bp8 = vec8("bp8", bp_d)
        qg8 = vec8("qg8", qg_d)
        qb8 = vec8("qb8", qb_d)
        kg8 = vec8("kg8", kg_d)
        kb8 = vec8("kb8", kb_d)

        # ---- constants ----
        ones_blk = const.tile([128, 128], F32, tag="onesblk")
        nc.vector.memset(ones_blk, 1.0)
        ones1b = const.tile([128, 1], BF16, tag="ones1b")
        nc.vector.tensor_copy(out=ones1b, in_=ones_blk[:, 0:1])
        ones1c = const.tile([128, 1], BF16, tag="ones1c")  # 1/C (exact in bf16)
        nc.vector.memset(ones1c, 1.0 / C)
        ones_row = const.tile([1, 128], BF16, tag="onesrow")
        nc.vector.tensor_copy(out=ones_row, in_=ones_blk[0:1, :])
        eps1 = const.tile([1, 1], F32)
        nc.vector.memset(eps1, EPS)
        scr1 = const.tile([1, 1], F32, tag="scr1")
        # dummy Sqrt at t=0 pre-loads the sqrt ACT table set off the critical path
        nc.scalar.activation(out=scr1, in_=eps1, func=AF.Sqrt, bias=eps1, scale=1.0)

        # persistent activations
        q_sb = qsbp.tile([128, NCH, TQ], BF16)      # q^T, later q-hat
        k_sb = ksbp.tile([128, NCH, T], BF16)       # k^T, later k-hat
        v_sb = vsbp.tile([128, NCH, H * 65], BF16)  # v head-interleaved + ones col
        outT_sb = osbp.tile([128, NCH, TQ], BF16)

        tmp_ctx = ExitStack()
        rows = tmp_ctx.enter_context(pool(name="rows", bufs=1))
        packp = tmp_ctx.enter_context(pool(name="pack", bufs=1))
        mrp = tmp_ctx.enter_context(pool(name="mr", bufs=1))
        sqp = tmp_ctx.enter_context(pool(name="sq", bufs=2))
        qsqp = tmp_ctx.enter_context(pool(name="qsq", bufs=1))

        def rowchain_half(sum_ap, sq_ap, mu_out, d_out):
            """mu_out = sum/C (bf16); d_out = sumsq - sum^2/C (f32), [1,512]."""
            nc.vector.tensor_scalar(out=mu_out, in0=sum_ap, scalar1=1.0 / C,
                                    scalar2=None, op0=OP.mult)
            t2 = rows.tile([1, 512], F32, tag="rt2")
            nc.vector.tensor_tensor(out=t2, in0=sum_ap, in1=mu_out, op=OP.mult)
            nc.vector.scalar_tensor_tensor(out=d_out, in0=t2, scalar=-1.0,
                                           in1=sq_ap, op0=OP.mult, op1=OP.add)

        def ln_exp(d_row, rs_out):
            """rs_out = 1/sqrt(d/C + eps): ACT Sqrt (sqrt stays the resident
            table set through all three LN phases) + DVE fast reciprocal."""
            nc.scalar.activation(out=d_row, in_=d_row, func=AF.Sqrt,
                                 bias=eps1, scale=1.0 / C)
            rcp = rows.tile([1, 1024], F32, tag="rrcp")
            n = d_row.shape[-1]
            nc.vector.reciprocal_approx_fast(out=rcp[:, 0:n], in_=d_row)
            nc.vector.tensor_copy(out=rs_out, in_=rcp[:, 0:n])

        def bc_half(mu_ap, rs_ap, bc_ps):
            """bc_ps[:, 0:512] = mu broadcast, [:, 512:1024] = rs broadcast."""
            nc.tensor.matmul(bc_ps[:, 0:512], ones_row, mu_ap, start=True, stop=True)
            nc.tensor.matmul(bc_ps[:, 512:1024], ones_row, rs_ap, start=True, stop=True)

        # ================= phase A: x stats, normalize =================
        # PSUM discipline: just two pools are live through phases A/B --
        # qmm (banks for q pass tiles) and kmm (k tiles). All stats /
        # broadcast / warm scratch scribbles into dead regions of the q pass
        # tiles (safe: every region's readers complete before the next
        # writer's accumulation group starts; the tile framework tracks the
        # range-level WAR/RAW dependencies).
        qmm_ctx = ExitStack()
        qmmp = qmm_ctx.enter_context(pool(name="qmm", bufs=1, space="PSUM"))
        kmm_ctx = ExitStack()
        kmmp = kmm_ctx.enter_context(pool(name="kmm", bufs=2, space="PSUM"))
        q_ps1 = qmmp.tile([128, 2048], F32, tag="mm")

        # tiny warm-up matmuls at t~0: absorb PE engine-start latency early
        for _ in range(3):
            nc.tensor.matmul(q_ps1[0:1, 1024:1025], ones1b, ones1b,
                             start=True, stop=True)

        # x stats: mean matmuls (ones=1/C) first, packed on partition 0 of
        # q_ps1 as mu0|mu1|ex0|ex1 so the rowchain runs 1024-wide
        xstat = q_ps1
        sqts = []
        for j in range(NCH):
            sqt = sqp.tile([128, T], BF16, tag="sqb")
            nc.vector.tensor_tensor(out=sqt, in0=xts[j], in1=xts[j], op=OP.mult)
            sqts.append(sqt)
        for j in range(NCH):
            st, sp = j == 0, j == NCH - 1
            nc.tensor.matmul(xstat[0:1, 0:512], ones1c, xts[j][:, 0:512],
                             start=st, stop=sp)
            nc.tensor.matmul(xstat[0:1, 512:1024], ones1c, xts[j][:, 512:1024],
                             start=st, stop=sp)
        for j in range(NCH):
            st, sp = j == 0, j == NCH - 1
            nc.tensor.matmul(xstat[0:1, 1024:1536], ones1c, sqts[j][:, 0:512],
                             start=st, stop=sp)
            nc.tensor.matmul(xstat[0:1, 1536:2048], ones1c, sqts[j][:, 512:1024],
                             start=st, stop=sp)

        mux = packp.tile([1, 1024], BF16, tag="mux")   # mu0 | mu1
        rsx = packp.tile([1, 1024], BF16, tag="rsx")   # rs0 | rs1
        nc.vector.tensor_copy(out=mux, in_=xstat[0:1, 0:1024])
        t2x = rows.tile([1, 1024], F32, tag="rt2x")
        nc.vector.tensor_tensor(out=t2x, in0=xstat[0:1, 0:1024], in1=mux, op=OP.mult)
        dx = rows.tile([1, 1024], F32, tag="rdx")
        nc.vector.scalar_tensor_tensor(out=dx, in0=t2x, scalar=-1.0,
                                       in1=xstat[0:1, 1024:2048], op0=OP.mult, op1=OP.add)
        nc.scalar.activation(out=dx, in_=dx, func=AF.Sqrt, bias=eps1, scale=1.0)
        rcpx = rows.tile([1, 1024], F32, tag="rrcpx")
        nc.vector.reciprocal_approx_fast(out=rcpx, in_=dx)
        nc.vector.tensor_copy(out=rsx, in_=rcpx)

        # warm matmul keyed on mux (written ~1us into the chain): keeps the
        # PE HAM activity window hot through the rowchain stall
        nc.tensor.matmul(q_ps1[:, 1024:1536], ones_row, mux[:, 0:512],
                         start=True, stop=True)

        mrx = mrp.tile([128, 2048], BF16)  # mu0|rs0|mu1|rs1 bf16 broadcast
        for h in range(2):
            bch = q_ps1[:, h * 1024:(h + 1) * 1024]
            bc_half(mux[:, h * 512:(h + 1) * 512], rsx[:, h * 512:(h + 1) * 512], bch)
            nc.vector.tensor_copy(out=mrx[:, h * 1024:h * 1024 + 512], in_=bch[:, 0:512])
            nc.vector.tensor_copy(out=mrx[:, h * 1024 + 512:h * 1024 + 1024],
                                  in_=bch[:, 512:1024])

        # normalize in place, half 0 (query cols) first; q pass-1 matmuls are
        # interleaved j-outer so the PE consumes chunks as they land; half 1
        # runs on GpSimd so it doesn't pace the k matmuls behind DVE work
        for j in range(NCH):
            tz = sqp.tile([128, T], BF16, tag="sqb")
            nc.vector.tensor_tensor(out=tz[:, 0:512], in0=xts[j][:, 0:512],
                                    in1=mrx[:, 0:512], op=OP.subtract)
            nc.vector.tensor_tensor(out=xts[j][:, 0:512], in0=tz[:, 0:512],
                                    in1=mrx[:, 512:1024], op=OP.mult)
            for m in range(4):
                nc.tensor.matmul(q_ps1[:, m * 512:(m + 1) * 512], wq_v[:, m, j, :],
                                 xts[j][:, 0:TQ], start=(j == 0), stop=(j == NCH - 1))
        for j in range(NCH):
            tz = sqp.tile([128, T], BF16, tag="sqb")
            nc.vector.tensor_tensor(out=tz[:, 0:512], in0=xts[j][:, 512:1024],
                                    in1=mrx[:, 1024:1536], op=OP.subtract)
            nc.vector.tensor_tensor(out=xts[j][:, 512:1024], in0=tz[:, 0:512],
                                    in1=mrx[:, 1536:2048], op=OP.mult)

        # ones columns of v (DVE, early; cols disjoint from v drains)
        v_ones_view = v_sb.rearrange("p i (h x) -> p i h x", x=65)[:, :, :, 64:65]
        nc.vector.tensor_copy(out=v_ones_view,
                              in_=ones_blk.rearrange("p (i h x) -> p i h x", i=NCH, h=H))

        # ================= phase B: q pass 2, q-LN, k, k-LN =================
        for m in range(4):
            nc.scalar.activation(out=q_sb[:, m, :], in_=q_ps1[:, m * 512:(m + 1) * 512],
                                 func=AF.Identity, bias=bq8[:, m:m + 1], scale=1.0)
        q_ps2 = qmmp.tile([128, 2048], F32, tag="mm")
        for j in range(NCH):
            for m in range(4):
                nc.tensor.matmul(q_ps2[:, m * 512:(m + 1) * 512], wq_v[:, 4 + m, j, :],
                                 xts[j][:, 0:TQ], start=(j == 0), stop=(j == NCH - 1))
        for m in range(4):
            nc.scalar.activation(out=q_sb[:, 4 + m, :], in_=q_ps2[:, m * 512:(m + 1) * 512],
                                 func=AF.Identity, bias=bq8[:, 4 + m:4 + m + 1], scale=1.0)

        # q squares on DVE; the stat ones-matmuls are deferred into the k
        # loop so the PE is not stalled waiting on q drains
        qsq = []
        for m in range(NCH):
            sqt = qsqp.tile([128, TQ], BF16, tag=f"qsq{m}")
            nc.vector.tensor_tensor(out=sqt, in0=q_sb[:, m, :], in1=q_sb[:, m, :],
                                    op=OP.mult)
            qsq.append(sqt)

        muq = packp.tile([1, 512], BF16, tag="muq")
        rsq = packp.tile([1, 512], BF16, tag="rsq")
        dq = rows.tile([1, 512], F32, tag="rdq")
        mrq = mrp.tile([128, 1024], BF16, tag="mrq")

        # scribble regions inside q_ps2 (its drains complete early in the k loop)
        qstat_ps = q_ps2     # [0/32, 0:512]
        kstat_ps = q_ps2     # [0/32/64/96, 1536:2048]

        def kstat_mm(m):
            sqt = sqp.tile([128, T], BF16, tag="ksq")
            nc.vector.tensor_tensor(out=sqt, in0=k_sb[:, m, :], in1=k_sb[:, m, :], op=OP.mult)
            st, sp = m == 0, m == NCH - 1
            nc.tensor.matmul(kstat_ps[0:1, 1536:2048], ones1b, k_sb[:, m, 0:512],
                             start=st, stop=sp, tile_position=(0, 0))
            nc.tensor.matmul(kstat_ps[32:33, 1536:2048], ones1b, k_sb[:, m, 512:1024],
                             start=st, stop=sp, tile_position=(0, 32))
            nc.tensor.matmul(kstat_ps[64:65, 1536:2048], ones1b, sqt[:, 0:512],
                             start=st, stop=sp, tile_position=(0, 64))
            nc.tensor.matmul(kstat_ps[96:97, 1536:2048], ones1b, sqt[:, 512:1024],
                             start=st, stop=sp, tile_position=(0, 96))

        for m in range(NCH):
            k_ps = kmmp.tile([128, T], F32, tag="mm")
            for n in range(2):
                for j in range(NCH):
                    nc.tensor.matmul(k_ps[:, n * 512:(n + 1) * 512], wk_v[:, m, j, :],
                                     xts[j][:, n * 512:(n + 1) * 512],
                                     start=(j == 0), stop=(j == NCH - 1))
            nc.scalar.activation(out=k_sb[:, m, :], in_=k_ps, func=AF.Identity,
                                 bias=bk8[:, m:m + 1], scale=1.0)
            if m == 1:
                # q stat ones-matmuls: q drains/squares are done by now
                for mm_ in range(NCH):
                    st, sp = mm_ == 0, mm_ == NCH - 1
                    nc.tensor.matmul(qstat_ps[0:1, 0:512], ones1b, q_sb[:, mm_, :],
                                     start=st, stop=sp, tile_position=(0, 0))
                    nc.tensor.matmul(qstat_ps[32:33, 0:512], ones1b, qsq[mm_],
                                     start=st, stop=sp, tile_position=(0, 32))
                rowchain_half(qstat_ps[0:1, 0:512], qstat_ps[32:33, 0:512], muq, dq)
                ln_exp(dq, rsq)
            if m == 2:
                bcq0 = q_ps2[:, 512:1536]
                bc_half(muq, rsq, bcq0)
                nc.vector.tensor_copy(out=mrq, in_=bcq0)
            if 2 <= m <= 5:
                # q-hat applies, two per k iteration (keeps ACT from backing
                # up the k drains that release the kmm psum banks)
                for mm_ in range(2 * (m - 2), 2 * (m - 2) + 2):
                    t1 = sqp.tile([128, T], BF16, tag="sqb")
                    nc.vector.tensor_tensor(out=t1[:, 0:TQ], in0=q_sb[:, mm_, :],
                                            in1=mrq[:, 0:512], op=OP.subtract)
                    t2 = sqp.tile([128, T], BF16, tag="sqb")
                    nc.vector.tensor_tensor(out=t2[:, 0:TQ], in0=t1[:, 0:TQ],
                                            in1=mrq[:, 512:1024], op=OP.mult)
                    nc.scalar.activation(out=q_sb[:, mm_, :], in_=t2[:, 0:TQ],
                                         func=AF.Identity, bias=qb8[:, mm_:mm_ + 1],
                                         scale=qg8[:, mm_:mm_ + 1])
            if m >= 3:
                # staggered k stats for chunk m-3 (its drain+square are done)
                kstat_mm(m - 3)
        for m in range(NCH - 3, NCH):
            kstat_mm(m)

        kmm_ctx.close()

        # ================= phase C: v matmuls, then attention =================
        vps_ctx = ExitStack()
        vpsp = vps_ctx.enter_context(pool(name="vps", bufs=2, space="PSUM"))

        muk = packp.tile([1, 1024], BF16, tag="muk")
        rsk = packp.tile([1, 1024], BF16, tag="rsk")
        dk = rows.tile([1, 1024], F32, tag="rdk")
        rowchain_half(kstat_ps[0:1, 1536:2048], kstat_ps[64:65, 1536:2048],
                      muk[:, 0:512], dk[:, 0:512])
        rowchain_half(kstat_ps[32:33, 1536:2048], kstat_ps[96:97, 1536:2048],
                      muk[:, 512:1024], dk[:, 512:1024])
        ln_exp(dk, rsk)
        # preload the exp ACT table set now -- the ACT queue reaches this
        # during the v matmuls, well before the attention exps need it
        nc.scalar.activation(out=scr1, in_=eps1, func=AF.Exp, scale=1.0)

        mrk = mrp.tile([128, 2048], BF16, tag="mrk")

        def vgroup(g):
            wvsl = wv_v[:, g]
            for i in range(NCH):
                v_ps = vpsp.tile([128, 256], F32, tag="vps")
                for j in range(NCH):
                    nc.tensor.matmul(v_ps, xts[j][:, i * 128:(i + 1) * 128],
                                     wvsl[:, j, :], start=(j == 0), stop=(j == NCH - 1))
                vout = v_sb.rearrange("p i (h x) -> p i h x", x=65)[:, i, g * 4:(g + 1) * 4, 0:64]
                nc.vector.tensor_copy(out=vout, in_=v_ps.rearrange("p (h x) -> p h x", x=64))

        def khat(m):
            t1 = sqp.tile([128, T], BF16, tag="sqb")
            nc.vector.tensor_tensor(out=t1, in0=k_sb[:, m, :],
                                    in1=mrk[:, 0:1024], op=OP.subtract)
            t2 = sqp.tile([128, T], BF16, tag="sqb")
            nc.vector.tensor_tensor(out=t2, in0=t1,
                                    in1=mrk[:, 1024:2048], op=OP.mult)
            nc.scalar.activation(out=k_sb[:, m, :], in_=t2,
                                 func=AF.Identity, bias=kb8[:, m:m + 1],
                                 scale=kg8[:, m:m + 1])

        vgroup(0)
        # k broadcast, hidden under the v matmuls; mrk packs [mu0|mu1|rs0|rs1]
        for h in range(2):
            bckh = q_ps2[:, h * 1024:(h + 1) * 1024]
            bc_half(muk[:, h * 512:(h + 1) * 512], rsk[:, h * 512:(h + 1) * 512], bckh)
            nc.vector.tensor_copy(out=mrk[:, h * 512:h * 512 + 512], in_=bckh[:, 0:512])
            nc.vector.tensor_copy(out=mrk[:, 1024 + h * 512:1024 + h * 512 + 512],
                                  in_=bckh[:, 512:1024])
        vgroup(1)
        khat(0)
        khat(1)
        vgroup(2)
        khat(2)
        khat(3)
        vgroup(3)
        for m in range(4, NCH):
            khat(m)

        vps_ctx.close()
        qmm_ctx.close()
        tmp_ctx.close()
        xz_ctx.close()

        # ---- attention: software-pipelined pairs ----
        att_ctx = ExitStack()
        pexpp = att_ctx.enter_context(pool(name="pexp", bufs=16))
        denp = att_ctx.enter_context(pool(name="den", bufs=1))
        rcbp = att_ctx.enter_context(pool(name="rcb", bufs=2))
        scp = att_ctx.enter_context(pool(name="sc", bufs=2, space="PSUM"))
        avp = att_ctx.enter_context(pool(name="av", bufs=4, space="PSUM"))

        p_tiles = {}
        av_tiles = {}

        def sc_group(m, i, av_mm=None):
            """scores+exp for pair m, chunk i; av_mm emits the interleaved
            av matmuls of the previous pair."""
            sc_ps = scp.tile([128, 1024], F32, tag="sc")
            nc.tensor.matmul(sc_ps[:, 0:512],
                             k_sb[0:64, m, i * 128:(i + 1) * 128],
                             q_sb[0:64, m, :], start=True, stop=True)
            nc.tensor.matmul(sc_ps[:, 512:1024],
                             k_sb[64:128, m, i * 128:(i + 1) * 128],
                             q_sb[64:128, m, :], start=True, stop=True)
            p_sb = pexpp.tile([128, 1024], BF16, tag="p")
            nc.scalar.activation(out=p_sb, in_=sc_ps, func=AF.Exp, scale=0.125)
            p_tiles[m].append(p_sb)
            if av_mm is not None:
                av_mm(i)

        def av_group_fn(mprev):
            p_list = p_tiles[mprev]
            h0, h1 = 2 * mprev, 2 * mprev + 1
            av0 = avp.tile([65, TQ], F32, tag="av")
            av1 = avp.tile([65, TQ], F32, tag="av")
            av_tiles[mprev] = (av0, av1)

            def av_mm(i):
                st, sp = i == 0, i == NCH - 1
                nc.tensor.matmul(av0, v_sb[:, i, h0 * 65:h0 * 65 + 65],
                                 p_list[i][:, 0:512], start=st, stop=sp)
                nc.tensor.matmul(av1, v_sb[:, i, h1 * 65:h1 * 65 + 65],
                                 p_list[i][:, 512:1024], start=st, stop=sp)
            return av_mm

        def av_drain(mprev):
            av0, av1 = av_tiles.pop(mprev)
            p_tiles.pop(mprev)
            dd = denp.tile([1, 2 * TQ], F32, tag="den")
            nc.vector.tensor_copy(out=dd[:, 0:TQ], in_=av0[64:65, :])
            nc.vector.tensor_copy(out=dd[:, TQ:2 * TQ], in_=av1[64:65, :])
            rt = denp.tile([1, 2 * TQ], F32, tag="rect")
            nc.vector.reciprocal_approx_fast(out=rt, in_=dd)
            rbb = rcbp.tile([64, 2 * TQ], F32, tag="rbb")
            nc.gpsimd.partition_broadcast(rbb, rt)
            nc.vector.tensor_tensor(out=outT_sb[0:64, mprev, :],
                                    in0=av0[0:64, :], in1=rbb[:, 0:TQ], op=OP.mult)
            nc.vector.tensor_tensor(out=outT_sb[64:128, mprev, :],
                                    in0=av1[0:64, :], in1=rbb[:, TQ:2 * TQ], op=OP.mult)

        p_tiles[0] = []
        for i in range(NCH):
            sc_group(0, i)
        for m in range(1, NCH):
            p_tiles[m] = []
            av_mm = av_group_fn(m - 1)
            for i in range(NCH):
                sc_group(m, i, av_mm)
            av_drain(m - 1)
        av_mm = av_group_fn(NCH - 1)
        for i in range(NCH):
            av_mm(i)
        av_drain(NCH - 1)

        att_ctx.close()

        # ================= phase D: proj =================
        youtp = ctx.enter_context(pool(name="yout", bufs=2))
        pjp = ctx.enter_context(pool(name="pj", bufs=2, space="PSUM"))
        for m in range(NCH):
            y_ps = pjp.tile([128, TQ], F32, tag="pj")
            for j in range(NCH):
                nc.tensor.matmul(y_ps, wp_v[:, j, m * 128:(m + 1) * 128], outT_sb[:, j, :],
                                 start=(j == 0), stop=(j == NCH - 1))
            y_sb = youtp.tile([128, TQ], F32, tag="y")
            nc.scalar.activation(out=y_sb, in_=y_ps, func=AF.Identity,
                                 bias=bp8[:, m:m + 1], scale=1.0)
            nc.sync.dma_start(out=yT_d[m * 128:(m + 1) * 128, :], in_=y_sb)

    nc.finalize()
    return nc


def _get_nc():
    if "nc" not in _CACHE:
        _CACHE["nc"] = _build()
    return _CACHE["nc"]


def _lay_w(w, gcols):
    """[C, C] -> [128, 8192] slab-contiguous: A[p, m, j, c'] = w[j*128+p, m*gcols+c']."""
    A = w.reshape(NCH, 128, C // gcols, gcols).transpose(1, 2, 0, 3)
    return np.ascontiguousarray(A.reshape(128, NCH * C))


def _prep_inputs(x, norm_g, norm_b, qkv_w, qkv_b, qln_g, qln_b, kln_g, kln_b, proj_w, proj_b):
    x = np.asarray(x, dtype=np.float32)
    norm_g = np.asarray(norm_g, dtype=np.float32)
    norm_b = np.asarray(norm_b, dtype=np.float32)
    qkv_w = np.asarray(qkv_w, dtype=np.float32)
    qkv_b = np.asarray(qkv_b, dtype=np.float32)
    proj_w = np.asarray(proj_w, dtype=np.float32)
    proj_b = np.asarray(proj_b, dtype=np.float32)

    wfold = norm_g[:, None] * qkv_w                    # [C, 3C]
    bfold = qkv_b + norm_b @ qkv_w                     # [3C]
    wq = np.ascontiguousarray(wfold[:, 0:C])
    wk = np.ascontiguousarray(wfold[:, C:2 * C])
    wv = np.ascontiguousarray(wfold[:, 2 * C:3 * C])
    bq, bk, bv = bfold[0:C].copy(), bfold[C:2 * C].copy(), bfold[2 * C:3 * C].copy()
    # v bias folds through attention (softmax rows sum to 1) into proj bias
    bp = proj_b + bv @ proj_w

    bf16 = ml_dtypes.bfloat16
    wp_lay = np.ascontiguousarray(
        proj_w.reshape(NCH, 128, C).transpose(1, 0, 2).reshape(128, NCH * C))
    common = dict(
        wq=_lay_w(wq, 128).astype(bf16), wk=_lay_w(wk, 128).astype(bf16),
        wv=_lay_w(wv, 256).astype(bf16), wp=wp_lay.astype(bf16),
        bq=bq, bk=bk, bp=bp,
        qg=np.asarray(qln_g, dtype=np.float32).copy(),
        qb=np.asarray(qln_b, dtype=np.float32).copy(),
        kg=np.asarray(kln_g, dtype=np.float32).copy(),
        kb=np.asarray(kln_b, dtype=np.float32).copy(),
    )
    in_maps = []
    for core in range(8):
        b, half = core // 2, core % 2
        xp = np.concatenate([x[b, TQ * half:], x[b, :TQ * half]], axis=0) if half else x[b]
        xT = np.ascontiguousarray(xp.T).astype(bf16)
        in_maps.append(dict(common, xT=xT))
    return in_maps


def kernel(**inputs) -> np.ndarray:
    in_maps = _prep_inputs(**inputs)
    nc = _get_nc()
    res = run_bass_kernel_spmd(nc, in_maps, core_ids=list(range(8)))
    out = np.empty((B, T, C), dtype=np.float32)
    for core in range(8):
        b, half = core // 2, core % 2
        out[b, TQ * half:TQ * half + TQ, :] = res.results[core]["yT"].T
    return out
